# revision 59
# baseline (speedup 1.0000x reference)
"""Trainium2 Bass kernel for the hybrid attention head (nn_AttentionHead_Hybrid).

Math (per batch):
    norms  n_i = ||x_i||;  xh = x / n
    O      = product of 2016 Givens rotations (built on host, fp32)
    S[i,j] = xh_i . O . xh_j
    A      = S^2 * n_i n_j ;  P = softmax(A / 8)
    V      = x @ Vw^T + Vb
    out    = LayerNorm(P @ V + x) * gamma + beta

Device formulation (per core, 4 batches):
    W'     = diag(s') X with s'_n = ||x_n||^-1/2 * 8^-1/4 * (128/ln2)^1/4
    gt     = (W' O)^T  (host-prepped, f16)          so R[j,i] = w'_j O^T w'_i
    R^2    = (128/ln2) * A/8
    E      = exp(A/8) via Schraudolph: bitcast_bf16(u16(R^2 + B)); squares on
             ACT (PSUM->SBUF f16), +B/u16-convert on DVE (4x tensor_scalar)
    Vt     = [X Vw^T | sum_d | 1]  (bf16; V_b folded into the residual xr)
    OUT    = E @ Vt computed directly: per (i-tile, j-tile) matmul with the
             E^T slice as the stationary operand -> accumulates [128, 66] in
             PSUM per i-tile (no transposes, no PSUM->SBUF staging)
    y      = OUT[:, :64]*(1/OUT[:,65]) + xr; LayerNorm via the free sum_d
             column (mean) + square/reduce (variance); normalize as one
             per-tile tensor_scalar with per-partition scalar APs
    out    bf16, converted to f32 on host.

Measured on 8xTRN2: 59.4us (baseline handed to this session: 64.6us).
Engine balance per batch: ACT 8 squares (~8.1us), DVE adds+epilogue (~8us),
PE S+PV (~8us incl. warm-up HAM ramp), GpSimd idle (tested: any streaming
work on it contends with DVE via the shared SBUF port).

Sharding: data-parallel over batch, 4 batches per core on 8 cores.
"""

import math

import numpy as np
import ml_dtypes

import concourse.bacc as bacc
import concourse.bass as bass
import concourse.tile as tile
from concourse import bass_utils, mybir

AF = mybir.ActivationFunctionType
ALU = mybir.AluOpType
DT = mybir.dt

B, N, D = 32, 1024, 64
NCORES = 8
BPC = B // NCORES          # batches per core
NT = N // 128              # 128-row tiles per batch

SCHRAUD_C4 = 128.0 / math.log(2.0)          # scale absorbed into W' (4th root)
SCHRAUD_B = 16256.0 - 128.0 * 0.0430        # bf16 exponent bias, sigma-centered
SCHRAUD_B8 = 60.0 - 4.0 * 0.0430            # fp8e5m2 exponent bias, centered
E_FP8 = False              # E in fp8e5m2 (tested: slower — u8 ts loses 4x mode)

# ---- engine assignment knobs (per-batch, indexed by j-tile) ----
SQ_DVE_JTS = ()            # tested: any DVE square stalls the in-order PV chain
N_WARMUP_MM = 24           # tiny matmuls at t=0 to warm the PE HAM clock gate
NORM_ENGINE = "ts"         # LN normalize: "ts" (DVE tensor_scalar), "pool", "dve_tt"


def _register_square_bias():
    """Register the fused (x^2 + bias) custom DVE op: one PSUM-read pass that
    squares, adds the Schraudolph constant, and u16-converts on write."""
    from concourse import dve_ops as dops
    from concourse.dve_spec import Spec, Src0, C0, C1, sq, lower as dve_lower
    from concourse.dve_uop import DveOpSpec

    for op in dops.OPS:
        if op.name == "SQUARE_BIAS_ANT":
            return op
    spec = Spec(
        body=sq(Src0) * C1 + C0,
        reference=lambda in0, in1, s0, s1, imm2: (
            in0.astype(np.float32) * in0.astype(np.float32) * s1 + s0
        ),
    )
    shas = {
        ver: DveOpSpec(name="SQUARE_BIAS_ANT", uops=dve_lower(spec, ver=ver)).sha(ver)
        for ver in ("v3", "v4")
    }
    op = dops.DveOp("SQUARE_BIAS_ANT", spec, subdim=False, uops_sha=shas)
    dops.OPS.append(op)
    dops.CUSTOM_DVE_SPECS[op.name] = op.spec
    dops._SUB_OPCODE_FOR_NAME[op.name] = dops._CUSTOM_DVE_ROW_BASE + len(dops.OPS) - 1
    return op


SQUARE_BIAS_OP = _register_square_bias()


def _build_orthogonal(phi: np.ndarray, d: int = D) -> np.ndarray:
    """fp32 replica of the reference jax.lax.scan Givens chain."""
    O = np.eye(d, dtype=np.float32)
    ii, jj = np.triu_indices(d, k=1)
    c = np.cos(phi.astype(np.float32))
    s = np.sin(phi.astype(np.float32))
    for k in range(len(phi)):
        i, j = int(ii[k]), int(jj[k])
        ri = O[i].copy()
        rj = O[j].copy()
        O[i] = c[k] * ri + s[k] * rj
        O[j] = -s[k] * ri + c[k] * rj
    return O


def _bc(ap: bass.AP, reps: int, inner: int) -> bass.AP:
    """Broadcast a [128, reps] AP to [128, reps, inner] with stride-0 inner."""
    return bass.AP(
        tensor=ap.tensor, offset=ap.offset,
        ap=[ap.ap[0], [ap.ap[1][0], reps], [0, inner]],
    )


def _cols(og: bass.AP, h, col: int, strided: bool) -> bass.AP:
    """APs into the [128, 2, 512] OUT accumulator.

    strided=True: [128, (2,) 4] picking column `col` of each 66-wide region.
    strided=False (h given): [128, 4, 64] the numerator block of half h.
    """
    if strided:
        base = og[:, :, col] if h is None else og[:, h, col]
        dims = ([[512, 2]] if h is None else []) + [[66, 4]]
        return bass.AP(tensor=base.tensor, offset=base.offset,
                       ap=[base.ap[0], *dims])
    base = og[:, h, col]
    return bass.AP(tensor=base.tensor, offset=base.offset,
                   ap=[base.ap[0], [66, 4], [1, 64]])


def _build_nc(apply_gamma_beta: bool):
    nc = bacc.Bacc("TRN2", target_bir_lowering=False)

    wt_t = nc.dram_tensor("wt", [BPC, D, N], DT.float16, kind="ExternalInput")
    gt_t = nc.dram_tensor("gt", [BPC, D, N], DT.float16, kind="ExternalInput")
    # host-pretransposed: [p, t, c] so each partition's DMA line is contiguous
    v_dt = DT.float8e4 if E_FP8 else DT.bfloat16
    v_t = nc.dram_tensor("v", [BPC, 128, NT, 66], v_dt, kind="ExternalInput")
    xr_t = nc.dram_tensor("xr", [BPC, 128, NT, D], DT.bfloat16, kind="ExternalInput")
    xs_t = nc.dram_tensor("xs", [BPC, 128, NT], DT.float32, kind="ExternalInput")
    gb_t = nc.dram_tensor("gb", [2, D], DT.float32, kind="ExternalInput")
    # output in [p, t, d] layout; host untransposes
    out_t = nc.dram_tensor("out", [BPC, 128, NT, D], DT.bfloat16, kind="ExternalOutput")

    with tile.TileContext(nc) as tc:
        with (
            tc.tile_pool(name="const", bufs=1) as constp,
            tc.tile_pool(name="loadp", bufs=2) as loadp,
            tc.tile_pool(name="asqp", bufs=3) as asqp,
            tc.tile_pool(name="ep", bufs=3) as ep,
            tc.tile_pool(name="yp", bufs=2) as yp,
            tc.tile_pool(name="outp", bufs=2) as outp,
            tc.tile_pool(name="statp", bufs=2) as statp,
            tc.tile_pool(name="psA", bufs=2, space="PSUM") as psA,
            tc.tile_pool(name="psO", bufs=2, space="PSUM") as psO,
        ):
            # ---- PE warm-up: dense tiny matmuls from t=0 so the HAM clock
            # gate reaches K=8/8 (~2x PE clock) before the real matmuls ----
            wz = constp.tile([128, 128], DT.bfloat16)
            nc.vector.memset(wz, 1.0)
            pr_warm = psA.tile([128, N], DT.float32, tag="pr", name="prw")
            for k in range(N_WARMUP_MM):
                nc.tensor.matmul(pr_warm[:, 0:128], lhsT=wz, rhs=wz,
                                 start=True, stop=True)

            def emit_loads(b):
                # quadrant (0,0) reads gt cols 0:512, quadrant (64,0) reads
                # 512:N -- the unused gt quadrants are never DMA'd.
                wt = loadp.tile([128, N], DT.float16, tag="wt")
                gt = loadp.tile([128, N], DT.float16, tag="gt")
                if b == 0:
                    # chunk so S(0) can start after ~100KB instead of ~500KB
                    nc.sync.dma_start(out=wt[0:D, 0:128], in_=wt_t[b][:, 0:128])
                    nc.sync.dma_start(
                        out=wt[D: 2 * D, 0:128], in_=wt_t[b][:, 0:128]
                    )
                    nc.sync.dma_start(out=gt[0:D, 0:512], in_=gt_t[b][:, 0:512])
                    nc.sync.dma_start(
                        out=gt[D: 2 * D, 512:N], in_=gt_t[b][:, 512:N]
                    )
                    nc.sync.dma_start(out=wt[0:D, 128:N], in_=wt_t[b][:, 128:N])
                    nc.sync.dma_start(
                        out=wt[D: 2 * D, 128:N], in_=wt_t[b][:, 128:N]
                    )
                else:
                    nc.sync.dma_start(out=wt[0:D, :], in_=wt_t[b])
                    nc.sync.dma_start(out=wt[D: 2 * D, :], in_=wt_t[b])
                    nc.sync.dma_start(out=gt[0:D, 0:512], in_=gt_t[b][:, 0:512])
                    nc.sync.dma_start(
                        out=gt[D: 2 * D, 512:N], in_=gt_t[b][:, 512:N]
                    )
                v_sb = loadp.tile([128, NT, 66], v_dt, tag="v")
                nc.sync.dma_start(out=v_sb, in_=v_t[b])
                xr = loadp.tile([128, NT, D], DT.bfloat16, tag="xr")
                nc.sync.dma_start(out=xr, in_=xr_t[b])
                xs = loadp.tile([128, NT], DT.float32, tag="xs")
                nc.sync.dma_start(out=xs, in_=xs_t[b])
                return wt, gt, v_sb, xr, xs

            state = {0: emit_loads(0)}
            gam_sb = bet_sb = None
            if apply_gamma_beta:
                gam_sb = constp.tile([128, D], DT.float32)
                nc.sync.dma_start(out=gam_sb, in_=gb_t[0, :].to_broadcast([128, D]))
                bet_sb = constp.tile([128, D], DT.float32)
                nc.sync.dma_start(out=bet_sb, in_=gb_t[1, :].to_broadcast([128, D]))
            # pull the ACT table switches into the idle prologue
            dummy = constp.tile([128, 1], DT.float32)
            nc.vector.memset(dummy, 1.0)
            nc.scalar.activation(dummy, dummy, AF.Square)
            nc.scalar.activation(dummy, dummy, AF.Sqrt)
            eps_sb = constp.tile([128, 1], DT.float32)
            nc.vector.memset(eps_sb, 1e-5)

            def emit_epilogue_grp(b, og, xr, xs, st, g, last=False):
                """og: [128, 2, 512] PSUM f32 accum; g: half = i-tiles 4g..

                last=True: shift the square/accum and normalize onto ACT,
                which idles in the kernel tail while DVE is the bottleneck.
                """
                (rcol, mus, mu, m2, ve, sd, rstd, mub, rstdb, ys2, y, ysq,
                 out_sb) = st
                q_sl = slice(4 * g, 4 * g + 4)
                nc.vector.reciprocal_approx_fast(
                    rcol[:, q_sl], _cols(og, g, D + 1, True)
                )
                # y = OUT * (1/rowsum) + xr
                nc.vector.tensor_tensor(
                    out=y[:, q_sl, :], in0=_cols(og, g, 0, False),
                    in1=_bc(rcol[:, q_sl], 4, D), op=ALU.mult,
                )
                nc.vector.tensor_tensor(
                    out=mus[:, q_sl], in0=_cols(og, g, D, True),
                    in1=rcol[:, q_sl], op=ALU.mult,
                )
                nc.vector.tensor_add(y[:, q_sl, :], y[:, q_sl, :], xr[:, q_sl, :])
                # variance: Sum y^2 then ve = ys2/D - mu^2 (+eps via sqrt bias)
                if last:
                    for q in range(4 * g, 4 * g + 4):
                        nc.scalar.activation(
                            ysq[:, q, :], y[:, q, :], AF.Square,
                            accum_out=ys2[:, q: q + 1],
                        )
                else:
                    nc.vector.tensor_mul(ysq[:, q_sl, :], y[:, q_sl, :],
                                         y[:, q_sl, :])
                    nc.vector.reduce_sum(ys2[:, q_sl], ysq[:, q_sl, :],
                                         axis=mybir.AxisListType.X)
                nc.vector.scalar_tensor_tensor(
                    out=mu[:, q_sl], in0=mus[:, q_sl], scalar=1.0 / D,
                    in1=xs[:, q_sl], op0=ALU.mult, op1=ALU.add,
                )
                nc.vector.tensor_mul(m2[:, q_sl], mu[:, q_sl], mu[:, q_sl])
                nc.vector.scalar_tensor_tensor(
                    out=ve[:, q_sl], in0=ys2[:, q_sl], scalar=1.0 / D,
                    in1=m2[:, q_sl], op0=ALU.mult, op1=ALU.subtract,
                )
                nc.scalar.activation(sd[:, q_sl], ve[:, q_sl], AF.Sqrt, bias=eps_sb)
                nc.vector.reciprocal_approx_fast(rstd[:, q_sl], sd[:, q_sl])
                if NORM_ENGINE == "ts":
                    # out = y*rstd - mu*rstd with per-partition scalar APs
                    nc.vector.scalar_tensor_tensor(
                        out=mub[:, q_sl], in0=mu[:, q_sl], scalar=-1.0,
                        in1=rstd[:, q_sl], op0=ALU.mult, op1=ALU.mult,
                    )
                    for q in range(4 * g, 4 * g + 4):
                        nc.vector.tensor_scalar(
                            out_sb[:, q, :], y[:, q, :],
                            scalar1=rstd[:, q: q + 1],
                            scalar2=mub[:, q: q + 1],
                            op0=ALU.mult, op1=ALU.add,
                        )
                else:
                    eng = nc.gpsimd if NORM_ENGINE == "pool" else nc.vector
                    eng.tensor_tensor(
                        out=out_sb[:, q_sl, :], in0=y[:, q_sl, :],
                        in1=_bc(mu[:, q_sl], 4, D), op=ALU.subtract,
                    )
                    eng.tensor_tensor(
                        out=out_sb[:, q_sl, :], in0=out_sb[:, q_sl, :],
                        in1=_bc(rstd[:, q_sl], 4, D), op=ALU.mult,
                    )
                if apply_gamma_beta:
                    for q in range(4 * g, 4 * g + 4):
                        nc.gpsimd.tensor_mul(out_sb[:, q, :], out_sb[:, q, :], gam_sb)
                        nc.gpsimd.tensor_add(out_sb[:, q, :], out_sb[:, q, :], bet_sb)
                nc.sync.dma_start(out=out_t[b, :, q_sl, :], in_=out_sb[:, q_sl, :])

            def make_stats():
                rcol = statp.tile([128, NT], DT.float32, tag="rcol")
                mus = statp.tile([128, NT], DT.float32, tag="mus")
                mu = statp.tile([128, NT], DT.float32, tag="mu")
                m2 = statp.tile([128, NT], DT.float32, tag="m2")
                ve = statp.tile([128, NT], DT.float32, tag="ve")
                sd = statp.tile([128, NT], DT.float32, tag="sd")
                rstd = statp.tile([128, NT], DT.float32, tag="rstd")
                mub = statp.tile([128, NT], DT.float32, tag="mub")
                rstdb = statp.tile([128, NT], DT.bfloat16, tag="rstdb")
                ys2 = statp.tile([128, NT], DT.float32, tag="ys2")
                y = yp.tile([128, NT, D], DT.bfloat16, tag="y")
                ysq = yp.tile([128, NT, D], DT.bfloat16, tag="ysq")
                out_sb = outp.tile([128, NT, D], DT.bfloat16, tag="o")
                return (rcol, mus, mu, m2, ve, sd, rstd, mub, rstdb, ys2, y,
                        ysq, out_sb)

            pending = None   # (batch, og, xr, xs)
            carry_pv = None  # prev batch's emit_pv closure for deferred tails
            for b in range(BPC):
                wt, gt, v_sb, xr, xs = state[b]

                og = psO.tile([128, 2, 512], DT.float32, tag="og", name="og")
                ebs = {}

                def emit_s(jt):
                    pr = psA.tile([128, N], DT.float32, tag="pr", name="pr")
                    nc.tensor.matmul(
                        pr[:, 0:512],
                        lhsT=wt[0:D, jt * 128: (jt + 1) * 128],
                        rhs=gt[0:D, 0:512],
                        tile_position=(0, 0),
                    )
                    nc.tensor.matmul(
                        pr[:, 512:N],
                        lhsT=wt[D: 2 * D, jt * 128: (jt + 1) * 128],
                        rhs=gt[D: 2 * D, 512:N],
                        tile_position=(64, 0),
                    )
                    return pr

                def emit_sq_exp(jt, pr):
                    e = ep.tile([128, N], DT.uint16, tag="e")
                    if jt in SQ_DVE_JTS:
                        # fused square*scale + Schraudolph bias, one PSUM pass
                        nc.vector._custom_dve(
                            SQUARE_BIAS_OP, out=e, in0=pr, s0=SCHRAUD_B, s1=1.0
                        )
                    else:
                        asq = asqp.tile([128, N], DT.float16, tag="asq")
                        nc.scalar.activation(asq, pr, AF.Square)
                        nc.vector.tensor_scalar(
                            e, asq, scalar1=SCHRAUD_B, scalar2=None, op0=ALU.add
                        )
                    ebs[jt] = e.bitcast(DT.bfloat16)

                def emit_sq_exp_pair(j0, pr0, j1, pr1):
                    # squares per-tile on ACT; ONE paired +B add on DVE
                    asq = asqp.tile([128, 2, N], DT.float16, tag="asq")
                    nc.scalar.activation(asq[:, 0, :], pr0, AF.Square)
                    nc.scalar.activation(asq[:, 1, :], pr1, AF.Square)
                    e = ep.tile([128, 2, N], DT.uint16, tag="e")
                    nc.vector.tensor_scalar(
                        e, asq, scalar1=SCHRAUD_B, scalar2=None, op0=ALU.add
                    )
                    eb = e.bitcast(DT.bfloat16)
                    ebs[j0] = eb[:, 0, :]
                    ebs[j1] = eb[:, 1, :]

                def emit_pv(pj, og=og, ebs=ebs, v_sb=v_sb):
                    # default-arg binding: the closure must keep THIS batch's
                    # state when called as a deferred carry in the next batch.
                    # start=True clears has_written bits for the WHOLE bank, so
                    # only the first matmul into each bank may set it; the
                    # other regions' first writes rely on per-element
                    # overwrite-where-bit-clear behavior.
                    for it in range(NT):
                        c0 = (it % 4) * 66
                        nc.tensor.matmul(
                            og[:, it // 4, c0: c0 + 66],
                            lhsT=ebs[pj][:, it * 128: (it + 1) * 128],
                            rhs=v_sb[:, pj, :],
                            start=(pj == 0 and it % 4 == 0),
                            stop=(pj == NT - 1),
                            skip_group_check=True,
                        )

                for q in range(4):
                    j0, j1 = 2 * q, 2 * q + 1
                    pr0 = emit_s(j0)
                    pr1 = emit_s(j1)
                    # paired [128,2048] adds tested: slower (PV chain latency)
                    emit_sq_exp(j0, pr0)
                    emit_sq_exp(j1, pr1)
                    if q >= 1:
                        emit_pv(j0 - 2)
                        emit_pv(j1 - 2)
                    if q == 1 and pending is not None:
                        pending_st = make_stats()
                        emit_epilogue_grp(*pending, pending_st, 0)
                    if q == 2 and pending is not None:
                        emit_epilogue_grp(*pending, pending_st, 1)
                        pending = None
                    if q == 2 and b + 1 < BPC:
                        state[b + 1] = emit_loads(b + 1)
                # deferring these past the next batch's S pairs tested: slower
                # (delays og completion and the epilogue chain behind it)
                emit_pv(NT - 2)
                emit_pv(NT - 1)

                pending = (b, og, xr, xs)

            # tail: split the two groups' variance work across DVE (g0) and
            # the otherwise-idle ACT (g1) so the final chains run in parallel
            st = make_stats()
            emit_epilogue_grp(*pending, st, 0)
            emit_epilogue_grp(*pending, st, 1, last=True)

    nc.compile()
    return nc


_NC_CACHE: dict = {}


def kernel(input1, V_w, V_b, phi, ln_gamma, ln_beta, _trace=False):
    input1 = np.ascontiguousarray(np.asarray(input1, dtype=np.float32))
    V_w = np.asarray(V_w, dtype=np.float32)
    V_b = np.asarray(V_b, dtype=np.float32)
    phi = np.asarray(phi, dtype=np.float32)
    ln_gamma = np.asarray(ln_gamma, dtype=np.float32)
    ln_beta = np.asarray(ln_beta, dtype=np.float32)

    apply_gb = not (np.all(ln_gamma == 1.0) and np.all(ln_beta == 0.0))

    if apply_gb not in _NC_CACHE:
        _NC_CACHE[apply_gb] = _build_nc(apply_gb)
    nc = _NC_CACHE[apply_gb]

    O = _build_orthogonal(phi)
    gb = np.ascontiguousarray(np.stack([ln_gamma, ln_beta]).astype(np.float32))

    # host prep: W' = x * ||x||^-1/2 * 8^-1/4 * (128/ln2)^1/4 (transposed f16),
    # gt = (W' O)^T, xr = x + V_b (bf16 residual w/ folded bias)
    ss = (input1.astype(np.float64) ** 2).sum(-1)
    s = (ss ** -0.25 * 8.0 ** -0.25 * SCHRAUD_C4 ** 0.25).astype(np.float32)
    w = input1 * s[..., None]
    wt_full = np.ascontiguousarray(w.transpose(0, 2, 1).astype(np.float16))
    gt_full = np.ascontiguousarray(
        (w @ O).transpose(0, 2, 1).astype(np.float16)
    )
    vfull = input1 @ V_w.T
    v_cat = np.concatenate(
        [
            vfull,
            vfull.sum(-1, keepdims=True),
            np.ones((B, N, 1), np.float32),
        ],
        axis=-1,
    ).astype(ml_dtypes.float8_e4m3 if E_FP8 else ml_dtypes.bfloat16)
    # [B, N(=t*128+p), c] -> [B, p, t, c] for contiguous per-partition DMA
    v_full = np.ascontiguousarray(
        v_cat.reshape(B, NT, 128, 66).transpose(0, 2, 1, 3)
    )
    xr_f32 = input1 + V_b
    xr_full = np.ascontiguousarray(
        xr_f32.astype(ml_dtypes.bfloat16).reshape(B, NT, 128, D).transpose(0, 2, 1, 3)
    )
    xs_full = np.ascontiguousarray(
        (xr_f32.sum(-1) / D).astype(np.float32).reshape(B, NT, 128).transpose(0, 2, 1)
    )

    in_maps = []
    for c in range(NCORES):
        sl = slice(c * BPC, (c + 1) * BPC)
        in_maps.append(
            {
                "wt": wt_full[sl],
                "gt": gt_full[sl],
                "v": v_full[sl],
                "xr": xr_full[sl],
                "xs": xs_full[sl],
                "gb": gb,
            }
        )

    res = bass_utils.run_bass_kernel_spmd(
        nc, in_maps, core_ids=list(range(NCORES)), trace=_trace
    )
    # out comes back [BPC, p, t, d] -> [BPC, t*128+p, d]
    out = np.concatenate(
        [
            res.results[c]["out"].transpose(0, 2, 1, 3).reshape(BPC, N, D)
            for c in range(NCORES)
        ],
        axis=0,
    ).astype(np.float32)
    if _trace:
        kernel._last_result = res
    return out


# revision 60
# speedup vs baseline: 1.0445x; 1.0445x over previous
"""Trainium2 Bass kernel for the hybrid attention head (nn_AttentionHead_Hybrid).

Math (per batch):
    norms  n_i = ||x_i||;  xh = x / n
    O      = product of 2016 Givens rotations (built on host, fp32)
    S[i,j] = xh_i . O . xh_j
    A      = S^2 * n_i n_j ;  P = softmax(A / 8)
    V      = x @ Vw^T + Vb
    out    = LayerNorm(P @ V + x) * gamma + beta

Device formulation (per core, 4 batches):
    W'     = diag(s') X with s'_n = ||x_n||^-1/2 * 8^-1/4 * (128/ln2)^1/4
    gt     = (W' O)^T  (host-prepped, f16)          so R[j,i] = w'_j O^T w'_i
    R^2    = (128/ln2) * A/8
    E      = exp(A/8) via Schraudolph: bitcast_bf16(u16(R^2 + B)); squares on
             ACT (PSUM->SBUF f16), +B/u16-convert on DVE (4x tensor_scalar)
    Vt     = [X Vw^T | sum_d | 1]  (bf16; V_b folded into the residual xr)
    OUT    = E @ Vt computed directly: per (i-tile, j-tile) matmul with the
             E^T slice as the stationary operand -> accumulates [128, 66] in
             PSUM per i-tile (no transposes, no PSUM->SBUF staging)
    y      = OUT[:, :64]*(1/OUT[:,65]) + xr; LayerNorm via the free sum_d
             column (mean) + square/reduce (variance); normalize as one
             per-tile tensor_scalar with per-partition scalar APs
    out    bf16, converted to f32 on host.

Measured on 8xTRN2: 59.4us (baseline handed to this session: 64.6us).
Engine balance per batch: ACT 8 squares (~8.1us), DVE adds+epilogue (~8us),
PE S+PV (~8us incl. warm-up HAM ramp), GpSimd idle (tested: any streaming
work on it contends with DVE via the shared SBUF port).

Sharding: data-parallel over batch, 4 batches per core on 8 cores.
"""

import math

import numpy as np
import ml_dtypes

import concourse.bacc as bacc
import concourse.bass as bass
import concourse.tile as tile
from concourse import bass_utils, mybir

AF = mybir.ActivationFunctionType
ALU = mybir.AluOpType
DT = mybir.dt

B, N, D = 32, 1024, 64
NCORES = 8
BPC = B // NCORES          # batches per core
NT = N // 128              # 128-row tiles per batch

SCHRAUD_C4 = 128.0 / math.log(2.0)          # scale absorbed into W' (4th root)
SCHRAUD_B = 16256.0 - 128.0 * 0.0430        # bf16 exponent bias, sigma-centered
SCHRAUD_B8 = 60.0 - 4.0 * 0.0430            # fp8e5m2 exponent bias, centered
E_FP8 = False              # E in fp8e5m2 (tested: slower — u8 ts loses 4x mode)

# ---- engine assignment knobs (per-batch, indexed by j-tile) ----
SQ_DVE_JTS = ()            # tested: any DVE square stalls the in-order PV chain
N_WARMUP_MM = 24           # tiny matmuls at t=0 to warm the PE HAM clock gate
NORM_ENGINE = "ts"         # LN normalize: "ts" (DVE tensor_scalar), "pool", "dve_tt"


def _register_square_bias():
    """Register the fused (x^2 + bias) custom DVE op: one PSUM-read pass that
    squares, adds the Schraudolph constant, and u16-converts on write."""
    from concourse import dve_ops as dops
    from concourse.dve_spec import Spec, Src0, C0, C1, sq, lower as dve_lower
    from concourse.dve_uop import DveOpSpec

    for op in dops.OPS:
        if op.name == "SQUARE_BIAS_ANT":
            return op
    spec = Spec(
        body=sq(Src0) * C1 + C0,
        reference=lambda in0, in1, s0, s1, imm2: (
            in0.astype(np.float32) * in0.astype(np.float32) * s1 + s0
        ),
    )
    shas = {
        ver: DveOpSpec(name="SQUARE_BIAS_ANT", uops=dve_lower(spec, ver=ver)).sha(ver)
        for ver in ("v3", "v4")
    }
    op = dops.DveOp("SQUARE_BIAS_ANT", spec, subdim=False, uops_sha=shas)
    dops.OPS.append(op)
    dops.CUSTOM_DVE_SPECS[op.name] = op.spec
    dops._SUB_OPCODE_FOR_NAME[op.name] = dops._CUSTOM_DVE_ROW_BASE + len(dops.OPS) - 1
    return op


SQUARE_BIAS_OP = _register_square_bias()


def _build_orthogonal(phi: np.ndarray, d: int = D) -> np.ndarray:
    """fp32 replica of the reference jax.lax.scan Givens chain."""
    O = np.eye(d, dtype=np.float32)
    ii, jj = np.triu_indices(d, k=1)
    c = np.cos(phi.astype(np.float32))
    s = np.sin(phi.astype(np.float32))
    for k in range(len(phi)):
        i, j = int(ii[k]), int(jj[k])
        ri = O[i].copy()
        rj = O[j].copy()
        O[i] = c[k] * ri + s[k] * rj
        O[j] = -s[k] * ri + c[k] * rj
    return O


def _bc(ap: bass.AP, reps: int, inner: int) -> bass.AP:
    """Broadcast a [128, reps] AP to [128, reps, inner] with stride-0 inner."""
    return bass.AP(
        tensor=ap.tensor, offset=ap.offset,
        ap=[ap.ap[0], [ap.ap[1][0], reps], [0, inner]],
    )


def _cols(og: bass.AP, h, col: int, strided: bool) -> bass.AP:
    """APs into the [128, 2, 512] OUT accumulator.

    strided=True: [128, (2,) 4] picking column `col` of each 66-wide region.
    strided=False (h given): [128, 4, 64] the numerator block of half h.
    """
    if strided:
        base = og[:, :, col] if h is None else og[:, h, col]
        dims = ([[512, 2]] if h is None else []) + [[66, 4]]
        return bass.AP(tensor=base.tensor, offset=base.offset,
                       ap=[base.ap[0], *dims])
    base = og[:, h, col]
    return bass.AP(tensor=base.tensor, offset=base.offset,
                   ap=[base.ap[0], [66, 4], [1, 64]])


def _build_nc(apply_gamma_beta: bool):
    nc = bacc.Bacc("TRN2", target_bir_lowering=False)

    wt_t = nc.dram_tensor("wt", [BPC, D, N], DT.float16, kind="ExternalInput")
    gt_t = nc.dram_tensor("gt", [BPC, D, N], DT.float16, kind="ExternalInput")
    # host-pretransposed: [p, t, c] so each partition's DMA line is contiguous
    v_dt = DT.float8e4 if E_FP8 else DT.bfloat16
    v_t = nc.dram_tensor("v", [BPC, 128, NT, 66], v_dt, kind="ExternalInput")
    xr_t = nc.dram_tensor("xr", [BPC, 128, NT, D], DT.bfloat16, kind="ExternalInput")
    xs_t = nc.dram_tensor("xs", [BPC, 128, NT], DT.float32, kind="ExternalInput")
    gb_t = nc.dram_tensor("gb", [2, D], DT.float32, kind="ExternalInput")
    # output in [p, t, d] layout; host untransposes
    out_t = nc.dram_tensor("out", [BPC, 128, NT, D], DT.bfloat16, kind="ExternalOutput")

    with tile.TileContext(nc) as tc:
        with (
            tc.tile_pool(name="const", bufs=1) as constp,
            tc.tile_pool(name="loadp", bufs=2) as loadp,
            tc.tile_pool(name="asqp", bufs=3) as asqp,
            tc.tile_pool(name="ep", bufs=3) as ep,
            tc.tile_pool(name="yp", bufs=2) as yp,
            tc.tile_pool(name="outp", bufs=2) as outp,
            tc.tile_pool(name="statp", bufs=2) as statp,
            tc.tile_pool(name="psA", bufs=2, space="PSUM") as psA,
            tc.tile_pool(name="psO", bufs=2, space="PSUM") as psO,
        ):
            # ---- PE warm-up: dense tiny matmuls from t=0 so the HAM clock
            # gate reaches K=8/8 (~2x PE clock) before the real matmuls ----
            wz = constp.tile([128, 128], DT.bfloat16)
            nc.vector.memset(wz, 1.0)
            pr_warm = psA.tile([128, N], DT.float32, tag="pr", name="prw")
            for k in range(N_WARMUP_MM):
                nc.tensor.matmul(pr_warm[:, 0:128], lhsT=wz, rhs=wz,
                                 start=True, stop=True)

            def emit_loads(b):
                # quadrant (0,0) reads gt cols 0:512, quadrant (64,0) reads
                # 512:N -- the unused gt quadrants are never DMA'd.
                wt = loadp.tile([128, N], DT.float16, tag="wt")
                gt = loadp.tile([128, N], DT.float16, tag="gt")
                if b == 0:
                    # chunk so S(0) can start after ~100KB instead of ~500KB
                    nc.sync.dma_start(out=wt[0:D, 0:128], in_=wt_t[b][:, 0:128])
                    nc.sync.dma_start(
                        out=wt[D: 2 * D, 0:128], in_=wt_t[b][:, 0:128]
                    )
                    nc.sync.dma_start(out=gt[0:D, 0:512], in_=gt_t[b][:, 0:512])
                    nc.sync.dma_start(
                        out=gt[D: 2 * D, 512:N], in_=gt_t[b][:, 512:N]
                    )
                    nc.sync.dma_start(out=wt[0:D, 128:N], in_=wt_t[b][:, 128:N])
                    nc.sync.dma_start(
                        out=wt[D: 2 * D, 128:N], in_=wt_t[b][:, 128:N]
                    )
                else:
                    nc.sync.dma_start(out=wt[0:D, :], in_=wt_t[b])
                    nc.sync.dma_start(out=wt[D: 2 * D, :], in_=wt_t[b])
                    nc.sync.dma_start(out=gt[0:D, 0:512], in_=gt_t[b][:, 0:512])
                    nc.sync.dma_start(
                        out=gt[D: 2 * D, 512:N], in_=gt_t[b][:, 512:N]
                    )
                v_sb = loadp.tile([128, NT, 66], v_dt, tag="v")
                nc.sync.dma_start(out=v_sb, in_=v_t[b])
                xr = loadp.tile([128, NT, D], DT.bfloat16, tag="xr")
                nc.sync.dma_start(out=xr, in_=xr_t[b])
                xs = loadp.tile([128, NT], DT.float32, tag="xs")
                nc.sync.dma_start(out=xs, in_=xs_t[b])
                return wt, gt, v_sb, xr, xs

            state = {0: emit_loads(0)}
            gam_sb = bet_sb = None
            if apply_gamma_beta:
                gam_sb = constp.tile([128, D], DT.float32)
                nc.sync.dma_start(out=gam_sb, in_=gb_t[0, :].to_broadcast([128, D]))
                bet_sb = constp.tile([128, D], DT.float32)
                nc.sync.dma_start(out=bet_sb, in_=gb_t[1, :].to_broadcast([128, D]))
            # pull the ACT table switches into the idle prologue
            dummy = constp.tile([128, 1], DT.float32)
            nc.vector.memset(dummy, 1.0)
            nc.scalar.activation(dummy, dummy, AF.Square)
            nc.scalar.activation(dummy, dummy, AF.Sqrt)
            eps_sb = constp.tile([128, 1], DT.float32)
            nc.vector.memset(eps_sb, 1e-5)

            def emit_epilogue_grp(b, og, xr, xs, st, g, last=False):
                """og: [128, 2, 512] PSUM f32 accum; g: half = i-tiles 4g..

                last=True: shift the square/accum and normalize onto ACT,
                which idles in the kernel tail while DVE is the bottleneck.
                """
                (rcol, mus, mu, m2, ve, sd, rstd, mub, rstdb, ys2, y, ysq,
                 out_sb) = st
                q_sl = slice(4 * g, 4 * g + 4)
                nc.vector.reciprocal_approx_fast(
                    rcol[:, q_sl], _cols(og, g, D + 1, True)
                )
                # y = OUT * (1/rowsum) + xr
                nc.vector.tensor_tensor(
                    out=y[:, q_sl, :], in0=_cols(og, g, 0, False),
                    in1=_bc(rcol[:, q_sl], 4, D), op=ALU.mult,
                )
                nc.vector.tensor_tensor(
                    out=mus[:, q_sl], in0=_cols(og, g, D, True),
                    in1=rcol[:, q_sl], op=ALU.mult,
                )
                nc.vector.tensor_add(y[:, q_sl, :], y[:, q_sl, :], xr[:, q_sl, :])
                # variance: Sum y^2 then ve = ys2/D - mu^2 (+eps via sqrt bias)
                if last:
                    for q in range(4 * g, 4 * g + 4):
                        nc.scalar.activation(
                            ysq[:, q, :], y[:, q, :], AF.Square,
                            accum_out=ys2[:, q: q + 1],
                        )
                else:
                    nc.vector.tensor_mul(ysq[:, q_sl, :], y[:, q_sl, :],
                                         y[:, q_sl, :])
                    nc.vector.reduce_sum(ys2[:, q_sl], ysq[:, q_sl, :],
                                         axis=mybir.AxisListType.X)
                nc.vector.scalar_tensor_tensor(
                    out=mu[:, q_sl], in0=mus[:, q_sl], scalar=1.0 / D,
                    in1=xs[:, q_sl], op0=ALU.mult, op1=ALU.add,
                )
                nc.vector.tensor_mul(m2[:, q_sl], mu[:, q_sl], mu[:, q_sl])
                nc.vector.scalar_tensor_tensor(
                    out=ve[:, q_sl], in0=ys2[:, q_sl], scalar=1.0 / D,
                    in1=m2[:, q_sl], op0=ALU.mult, op1=ALU.subtract,
                )
                nc.scalar.activation(sd[:, q_sl], ve[:, q_sl], AF.Sqrt, bias=eps_sb)
                nc.vector.reciprocal_approx_fast(rstd[:, q_sl], sd[:, q_sl])
                if NORM_ENGINE == "ts":
                    # out = y*rstd - mu*rstd with per-partition scalar APs
                    nc.vector.scalar_tensor_tensor(
                        out=mub[:, q_sl], in0=mu[:, q_sl], scalar=-1.0,
                        in1=rstd[:, q_sl], op0=ALU.mult, op1=ALU.mult,
                    )
                    for q in range(4 * g, 4 * g + 4):
                        nc.vector.tensor_scalar(
                            out_sb[:, q, :], y[:, q, :],
                            scalar1=rstd[:, q: q + 1],
                            scalar2=mub[:, q: q + 1],
                            op0=ALU.mult, op1=ALU.add,
                        )
                else:
                    eng = nc.gpsimd if NORM_ENGINE == "pool" else nc.vector
                    eng.tensor_tensor(
                        out=out_sb[:, q_sl, :], in0=y[:, q_sl, :],
                        in1=_bc(mu[:, q_sl], 4, D), op=ALU.subtract,
                    )
                    eng.tensor_tensor(
                        out=out_sb[:, q_sl, :], in0=out_sb[:, q_sl, :],
                        in1=_bc(rstd[:, q_sl], 4, D), op=ALU.mult,
                    )
                if apply_gamma_beta:
                    for q in range(4 * g, 4 * g + 4):
                        nc.gpsimd.tensor_mul(out_sb[:, q, :], out_sb[:, q, :], gam_sb)
                        nc.gpsimd.tensor_add(out_sb[:, q, :], out_sb[:, q, :], bet_sb)
                nc.sync.dma_start(out=out_t[b, :, q_sl, :], in_=out_sb[:, q_sl, :])

            def make_stats():
                rcol = statp.tile([128, NT], DT.float32, tag="rcol")
                mus = statp.tile([128, NT], DT.float32, tag="mus")
                mu = statp.tile([128, NT], DT.float32, tag="mu")
                m2 = statp.tile([128, NT], DT.float32, tag="m2")
                ve = statp.tile([128, NT], DT.float32, tag="ve")
                sd = statp.tile([128, NT], DT.float32, tag="sd")
                rstd = statp.tile([128, NT], DT.float32, tag="rstd")
                mub = statp.tile([128, NT], DT.float32, tag="mub")
                rstdb = statp.tile([128, NT], DT.bfloat16, tag="rstdb")
                ys2 = statp.tile([128, NT], DT.float32, tag="ys2")
                y = yp.tile([128, NT, D], DT.bfloat16, tag="y")
                ysq = yp.tile([128, NT, D], DT.bfloat16, tag="ysq")
                out_sb = outp.tile([128, NT, D], DT.bfloat16, tag="o")
                return (rcol, mus, mu, m2, ve, sd, rstd, mub, rstdb, ys2, y,
                        ysq, out_sb)

            pending = None   # (batch, og, xr, xs)
            carry_pv = None  # prev batch's emit_pv closure for deferred tails
            for b in range(BPC):
                wt, gt, v_sb, xr, xs = state[b]

                og = psO.tile([128, 2, 512], DT.float32, tag="og", name="og")
                ebs = {}

                def emit_s(jt):
                    pr = psA.tile([128, N], DT.float32, tag="pr", name="pr")
                    nc.tensor.matmul(
                        pr[:, 0:512],
                        lhsT=wt[0:D, jt * 128: (jt + 1) * 128],
                        rhs=gt[0:D, 0:512],
                        tile_position=(0, 0),
                    )
                    nc.tensor.matmul(
                        pr[:, 512:N],
                        lhsT=wt[D: 2 * D, jt * 128: (jt + 1) * 128],
                        rhs=gt[D: 2 * D, 512:N],
                        tile_position=(64, 0),
                    )
                    return pr

                def emit_sq_exp(jt, pr):
                    e = ep.tile([128, N], DT.uint16, tag="e")
                    if jt in SQ_DVE_JTS:
                        # fused square*scale + Schraudolph bias, one PSUM pass
                        nc.vector._custom_dve(
                            SQUARE_BIAS_OP, out=e, in0=pr, s0=SCHRAUD_B, s1=1.0
                        )
                    else:
                        asq = asqp.tile([128, N], DT.float16, tag="asq")
                        nc.scalar.activation(asq, pr, AF.Square)
                        nc.vector.tensor_scalar(
                            e, asq, scalar1=SCHRAUD_B, scalar2=None, op0=ALU.add
                        )
                    ebs[jt] = e.bitcast(DT.bfloat16)

                def emit_sq_exp_pair(j0, pr0, j1, pr1):
                    # squares per-tile on ACT; ONE paired +B add on DVE
                    asq = asqp.tile([128, 2, N], DT.float16, tag="asq")
                    nc.scalar.activation(asq[:, 0, :], pr0, AF.Square)
                    nc.scalar.activation(asq[:, 1, :], pr1, AF.Square)
                    e = ep.tile([128, 2, N], DT.uint16, tag="e")
                    nc.vector.tensor_scalar(
                        e, asq, scalar1=SCHRAUD_B, scalar2=None, op0=ALU.add
                    )
                    eb = e.bitcast(DT.bfloat16)
                    ebs[j0] = eb[:, 0, :]
                    ebs[j1] = eb[:, 1, :]

                def emit_pv(pj, og=og, ebs=ebs, v_sb=v_sb):
                    # default-arg binding: the closure must keep THIS batch's
                    # state when called as a deferred carry in the next batch.
                    # start=True clears has_written bits for the WHOLE bank, so
                    # only the first matmul into each bank may set it; the
                    # other regions' first writes rely on per-element
                    # overwrite-where-bit-clear behavior.
                    for it in range(NT):
                        c0 = (it % 4) * 66
                        nc.tensor.matmul(
                            og[:, it // 4, c0: c0 + 66],
                            lhsT=ebs[pj][:, it * 128: (it + 1) * 128],
                            rhs=v_sb[:, pj, :],
                            start=(pj == 0 and it % 4 == 0),
                            stop=(pj == NT - 1),
                            skip_group_check=True,
                        )

                for q in range(4):
                    j0, j1 = 2 * q, 2 * q + 1
                    pr0 = emit_s(j0)
                    pr1 = emit_s(j1)
                    # paired [128,2048] adds tested: slower (PV chain latency)
                    emit_sq_exp(j0, pr0)
                    emit_sq_exp(j1, pr1)
                    if q >= 1:
                        emit_pv(j0 - 2)
                        emit_pv(j1 - 2)
                    if q == 1 and pending is not None:
                        pending_st = make_stats()
                        emit_epilogue_grp(*pending, pending_st, 0)
                    if q == 2 and pending is not None:
                        emit_epilogue_grp(*pending, pending_st, 1)
                        pending = None
                    if q == 2 and b + 1 < BPC:
                        state[b + 1] = emit_loads(b + 1)
                # deferring these past the next batch's S pairs tested: slower
                # (delays og completion and the epilogue chain behind it)
                emit_pv(NT - 2)
                emit_pv(NT - 1)

                pending = (b, og, xr, xs)

            # tail ACT-offload variants tested (full + variance-only): both
            # slower -- ACT's Square+accum/Identity ops cost 1.5-2x DVE's
            st = make_stats()
            emit_epilogue_grp(*pending, st, 0)
            emit_epilogue_grp(*pending, st, 1)

    nc.compile()
    return nc


_NC_CACHE: dict = {}


def kernel(input1, V_w, V_b, phi, ln_gamma, ln_beta, _trace=False):
    input1 = np.ascontiguousarray(np.asarray(input1, dtype=np.float32))
    V_w = np.asarray(V_w, dtype=np.float32)
    V_b = np.asarray(V_b, dtype=np.float32)
    phi = np.asarray(phi, dtype=np.float32)
    ln_gamma = np.asarray(ln_gamma, dtype=np.float32)
    ln_beta = np.asarray(ln_beta, dtype=np.float32)

    apply_gb = not (np.all(ln_gamma == 1.0) and np.all(ln_beta == 0.0))

    if apply_gb not in _NC_CACHE:
        _NC_CACHE[apply_gb] = _build_nc(apply_gb)
    nc = _NC_CACHE[apply_gb]

    O = _build_orthogonal(phi)
    gb = np.ascontiguousarray(np.stack([ln_gamma, ln_beta]).astype(np.float32))

    # host prep: W' = x * ||x||^-1/2 * 8^-1/4 * (128/ln2)^1/4 (transposed f16),
    # gt = (W' O)^T, xr = x + V_b (bf16 residual w/ folded bias)
    ss = (input1.astype(np.float64) ** 2).sum(-1)
    s = (ss ** -0.25 * 8.0 ** -0.25 * SCHRAUD_C4 ** 0.25).astype(np.float32)
    w = input1 * s[..., None]
    wt_full = np.ascontiguousarray(w.transpose(0, 2, 1).astype(np.float16))
    gt_full = np.ascontiguousarray(
        (w @ O).transpose(0, 2, 1).astype(np.float16)
    )
    vfull = input1 @ V_w.T
    v_cat = np.concatenate(
        [
            vfull,
            vfull.sum(-1, keepdims=True),
            np.ones((B, N, 1), np.float32),
        ],
        axis=-1,
    ).astype(ml_dtypes.float8_e4m3 if E_FP8 else ml_dtypes.bfloat16)
    # [B, N(=t*128+p), c] -> [B, p, t, c] for contiguous per-partition DMA
    v_full = np.ascontiguousarray(
        v_cat.reshape(B, NT, 128, 66).transpose(0, 2, 1, 3)
    )
    xr_f32 = input1 + V_b
    xr_full = np.ascontiguousarray(
        xr_f32.astype(ml_dtypes.bfloat16).reshape(B, NT, 128, D).transpose(0, 2, 1, 3)
    )
    xs_full = np.ascontiguousarray(
        (xr_f32.sum(-1) / D).astype(np.float32).reshape(B, NT, 128).transpose(0, 2, 1)
    )

    in_maps = []
    for c in range(NCORES):
        sl = slice(c * BPC, (c + 1) * BPC)
        in_maps.append(
            {
                "wt": wt_full[sl],
                "gt": gt_full[sl],
                "v": v_full[sl],
                "xr": xr_full[sl],
                "xs": xs_full[sl],
                "gb": gb,
            }
        )

    res = bass_utils.run_bass_kernel_spmd(
        nc, in_maps, core_ids=list(range(NCORES)), trace=_trace
    )
    # out comes back [BPC, p, t, d] -> [BPC, t*128+p, d]
    out = np.concatenate(
        [
            res.results[c]["out"].transpose(0, 2, 1, 3).reshape(BPC, N, D)
            for c in range(NCORES)
        ],
        axis=0,
    ).astype(np.float32)
    if _trace:
        kernel._last_result = res
    return out
